# revision 24
# baseline (speedup 1.0000x reference)
"""CapsNet dynamic-routing layer on 8 Trainium2 NeuronCores.

Math (per example, S=512 input capsules of dim D=256, 16 output capsules of
dim 32, O = 16*32 = 512):
  u_hat = x @ W                     # [S, O]
  b = 0; for 3 routing iters:
    c = softmax_n(b)                # over the 16-capsule axis
    s[n] = sum_s c[n,s] * u_hat[s, n*32:(n+1)*32]
    v = s / sqrt(|s|^2 + 1e-7)
    b[n,s] = v[n] . u_hat[s, n*32:(n+1)*32]
  out = v.flatten()

Sharding: pure data-parallel over the batch (256 examples -> 32 per core),
W replicated, no cross-core communication.

Per-core structure: examples are processed in groups of 4 so that all the
thin [16, *] routing tensors pack into 32-partition strips of full
128-partition tiles (strip j holds example 4g+j; rows 16..31 of each strip
are dead). The four per-example routing matmuls of a K-tile go to four
different PE column groups (tile_position=(0, 32j)) and run concurrently.

Layouts (per example):
  u_hat  [S, O]  (S on partitions, 4 tiles) - rhs of the s-matmul
  u_hatT [O, S]  (O on partitions, 4 tiles) - rhs of the b-update matmul
Both come straight off the tensor engine from xT = x.T (host-transposed)
since both contract over D. b is kept transposed as bT [S, 16-per-ex] so
softmax runs along the free dim. Matmul operands use float32r (single-pass
fp32, ~1.6e-4 relative) unless use_f32r=False.
"""

import sys

sys.path.insert(0, "/opt/trn_rl_repo")

import numpy as np

import concourse.bacc as bacc
import concourse.mybir as mybir
import concourse.tile as tile
from concourse import bass
from concourse.bass_utils import run_bass_kernel_spmd
from concourse.masks import make_identity

F32 = mybir.dt.float32
F32R = mybir.dt.float32r
U32 = mybir.dt.uint32
AX = mybir.AxisListType
AF = mybir.ActivationFunctionType
OP = mybir.AluOpType

B, S, D = 256, 512, 256
NC_, DC = 16, 32  # num_capsule, dim_capsule
O = NC_ * DC  # 512
N_CORES = 8
E = B // N_CORES  # 32 examples per core
G = 4  # examples per group (one per PE column-group strip)
ROUTINGS = 3
KT_D = D // 128  # 2 k-tiles over D
MT = 4  # 4 tiles over S and over O
RDT = F32R  # matmul operand dtype (set by build())
QMAGIC = 0x5F3759DF  # rsqrt seed magic


def host_masks():
    # dmask4[32j+n, n'*32+d] = (n' == n) for n < 16, else 0 (strip pads dead)
    dmask4 = np.zeros((128, O), np.float32)
    for j in range(G):
        for n in range(NC_):
            dmask4[32 * j + n, n * DC : (n + 1) * DC] = 1.0
    # vmaskL[q, j*64 + k*16 + n'] = (n' == 4k + q//32), j-independent
    vmaskL = np.zeros((128, G * 4 * NC_), np.float32)
    for q in range(128):
        for j in range(G):
            for k in range(4):
                vmaskL[q, j * 64 + k * NC_ + 4 * k + q // 32] = 1.0
    return dmask4, vmaskL


def emit_creation(nc, pools, consts, xT_ap, g, uh, uhT):
    (xp, up, utp, sp, ctp, pcre, pps, pet, pvt) = pools
    (W_t, dmask_t, vmask_t, ident_t, magic_t, c0_t) = consts

    # ---- load xT for 4 examples: [D, (e, S)] as 2 partition tiles ----
    xt = []
    for k in range(KT_D):
        t = xp.tile([128, G, S], RDT, tag=f"xt{k}")
        nc.sync.dma_start(
            t[:],
            xT_ap[G * g : G * (g + 1), 128 * k : 128 * (k + 1), :].rearrange(
                "e p s -> p e s"
            ),
        )
        xt.append(t)

    # ---- u_hat [S, O] and u_hatT [O, S] per example ----
    for j in range(G):
        for m in range(MT):
            pu = pcre.tile([128, O], F32, tag="pcre")
            for k in range(KT_D):
                nc.tensor.matmul(
                    pu[:],
                    xt[k][:, j, bass.ts(m, 128)],
                    W_t[k][:],
                    start=(k == 0),
                    stop=(k == KT_D - 1),
                )
            t = up.tile([128, O], RDT, tag=f"uh{j}{m}")
            nc.scalar.copy(t[:], pu[:])
            uh[j][m] = t
        for m in range(MT):
            pu = pcre.tile([128, S], F32, tag="pcre")
            for k in range(KT_D):
                nc.tensor.matmul(
                    pu[:],
                    W_t[k][:, bass.ts(m, 128)],
                    xt[k][:, j, :],
                    start=(k == 0),
                    stop=(k == KT_D - 1),
                )
            t = utp.tile([128, S], RDT, tag=f"uht{j}{m}")
            nc.scalar.copy(t[:], pu[:])
            uhT[j][m] = t
        yield


def emit_routing(nc, pools, consts, out_ap, g, uh, uhT):
    (xp, up, utp, sp, ctp, pcre, pps, pet, pvt) = pools
    (W_t, dmask_t, vmask_t, ident_t, magic_t, c0_t) = consts

    # ---- routing (4 examples packed in 32-partition strips) ----
    cT = None  # [S-chunk m] -> [128, (j,16)] compact; iter 0 uses 1/16 const
    v = None
    for it in range(ROUTINGS):
        # s_full_j = cT_j.T @ u_hat_j : own [16, O] psum per example, then a
        # fused mask+gather packs the strips into one [128, O] sbuf tile
        # (fp32-family matmuls cannot write PSUM at partition offset != 0)
        masked = sp.tile([128, O], F32, tag="masked")
        nc.gpsimd.memset(masked[:], 0.0)
        for j in range(G):
            ps = pps.tile([NC_, O], F32, tag="ps")
            for m in range(MT):
                lhs = (
                    c0_t[:]
                    if cT is None
                    else cT[m][:, NC_ * j : NC_ * (j + 1)]
                )
                nc.tensor.matmul(
                    ps[:],
                    lhs,
                    uh[j][m][:],
                    start=(m == 0),
                    stop=(m == MT - 1),
                )
            nc.vector.tensor_mul(
                masked[32 * j : 32 * j + NC_, :], ps[:], dmask_t[: NC_, :]
            )
        yield
        s = sp.tile([128, DC], F32, tag="s")
        nc.vector.tensor_reduce(
            s[:],
            masked[:].rearrange("p (n d) -> p d n", n=NC_),
            axis=AX.X,
            op=OP.add,
        )
        # squash: v = s * rsqrt(|s|^2 + 1e-7); rsqrt = quake seed + 3 Newton
        sq = sp.tile([128, DC], F32, tag="sq")
        ss = sp.tile([128, 1], F32, tag="ss")
        nc.scalar.activation(sq[:], s[:], AF.Square, accum_out=ss[:])
        q = sp.tile([128, 1], F32, tag="q")
        nc.vector.tensor_scalar_add(q[:], ss[:], 1e-7)
        sh = sp.tile([128, 1], U32, tag="sh")
        nc.vector.tensor_scalar(
            sh[:], q[:].bitcast(U32), 1, None, op0=OP.logical_shift_right
        )
        y = sp.tile([128, 1], F32, tag="y")
        nc.vector.tensor_tensor(
            y[:].bitcast(U32), magic_t[:], sh[:], op=OP.subtract
        )
        for _ in range(2):
            t2 = sp.tile([128, 1], F32, tag="t2")
            nc.vector.tensor_tensor(t2[:], y[:], y[:], op=OP.mult)
            nc.vector.tensor_tensor(t2[:], t2[:], q[:], op=OP.mult)
            nc.vector.tensor_scalar(
                t2[:], t2[:], -0.5, 1.5, op0=OP.mult, op1=OP.add
            )
            nc.vector.tensor_tensor(y[:], y[:], t2[:], op=OP.mult)
        v = sp.tile([128, DC], F32, tag="v")
        nc.vector.tensor_scalar_mul(v[:], s[:], y[:])

        if it == ROUTINGS - 1:
            break

        # ---- b update: bT'[strip j] = Vblk_j.T @ u_hatT_j ----
        vtp = pvt.tile([DC, 128], F32, tag="vtp")
        nc.tensor.transpose(vtp[:], v[:], ident_t[:])
        vv = sp.tile([128, G * 4], F32, tag="vv")
        vtp_jx = vtp[:].rearrange("p (j x) -> p j x", j=G)
        for r in range(4):  # strip row n_lo = r: VV[32r+d,(j,k)] = vtp[d,32j+4k+r]
            nc.vector.tensor_copy(
                vv[32 * r : 32 * (r + 1), :].rearrange("p (j k) -> p j k", j=G),
                vtp_jx[:, :, r : NC_ : 4],
            )
        vblk = sp.tile([128, G * 4 * NC_], RDT, tag="vblk")
        nc.vector.tensor_mul(
            vblk[:].rearrange("p (j k n) -> p j k n", j=G, k=4),
            vmask_t[:].rearrange("p (j k n) -> p j k n", j=G, k=4),
            vv[:]
            .rearrange("p (j k one) -> p j k one", j=G, one=1)
            .to_broadcast([128, G, 4, NC_]),
        )
        # bT'_j = Vblk_j.T @ u_hatT_j in its own [16, S] psum; the per-strip
        # exp packs results into one [128, S] sbuf tile
        expb = sp.tile([128, S], F32, tag="expb")
        nc.gpsimd.memset(expb[:], 0.0)
        for j in range(G):
            pb = pps.tile([NC_, S], F32, tag="ps")
            for k in range(MT):
                nc.tensor.matmul(
                    pb[:],
                    vblk[:, 64 * j + NC_ * k : 64 * j + NC_ * (k + 1)],
                    uhT[j][k][:],
                    start=(k == 0),
                    stop=(k == MT - 1),
                )
            nc.scalar.activation(
                expb[32 * j : 32 * j + NC_, :], pb[:], AF.Exp
            )
        yield
        et = pet.tile([128, MT, 128], F32, tag="et")
        r_all = sp.tile([128, MT * G], F32, tag="r_all")
        for m in range(MT):
            nc.tensor.transpose(et[:, m, :], expb[:, bass.ts(m, 128)], ident_t[:])
            nc.vector.tensor_reduce(
                r_all[:, G * m : G * (m + 1)],
                et[:, m, :].rearrange("p (j n) -> p j n", j=G)[:, :, :NC_],
                axis=AX.X,
                op=OP.add,
            )
        rinv = sp.tile([128, MT * G], F32, tag="rinv")
        nc.vector.reciprocal(rinv[:], r_all[:])
        cT = []
        for m in range(MT):
            ct = ctp.tile([128, G * NC_], RDT, tag=f"ct{m}")
            nc.vector.tensor_mul(
                ct[:].rearrange("p (j n) -> p j n", j=G),
                et[:, m, :].rearrange("p (j n) -> p j n", j=G)[:, :, :NC_],
                rinv[:, G * m : G * (m + 1)]
                .rearrange("p (j one) -> p j one", one=1)
                .to_broadcast([128, G, NC_]),
            )
            cT.append(ct)

    # ---- output: strip j -> row 4g+j ----
    for j in range(G):
        nc.sync.dma_start(
            out_ap[G * g + j].rearrange("(n d) -> n d", n=NC_),
            v[32 * j : 32 * j + NC_, :],
        )


def build(n_ex=E, num_devices=N_CORES, use_f32r=True):
    global RDT
    RDT = F32R if use_f32r else F32
    assert n_ex % G == 0
    nc = bacc.Bacc(
        "TRN2", target_bir_lowering=False, debug=False, num_devices=num_devices
    )
    xT_d = nc.dram_tensor("xT", [n_ex, D, S], RDT, kind="ExternalInput")
    W_d = nc.dram_tensor("W", [D, O], RDT, kind="ExternalInput")
    dmask_d = nc.dram_tensor("dmask", [128, O], F32, kind="ExternalInput")
    vmask_d = nc.dram_tensor("vmask", [128, G * 4 * NC_], F32, kind="ExternalInput")
    out_d = nc.dram_tensor("out", [n_ex, O], F32, kind="ExternalOutput")

    with tile.TileContext(nc) as tc:
        with (
            tc.tile_pool(name="consts", bufs=1) as cp,
            tc.tile_pool(name="xp", bufs=2) as xp,
            tc.tile_pool(name="up", bufs=2) as up,
            tc.tile_pool(name="utp", bufs=2) as utp,
            tc.tile_pool(name="sp", bufs=4) as sp,
            tc.tile_pool(name="ctp", bufs=3) as ctp,
            tc.tile_pool(name="pcre", bufs=2, space=bass.MemorySpace.PSUM) as pcre,
            tc.tile_pool(name="pps", bufs=3, space=bass.MemorySpace.PSUM) as pps,
            tc.tile_pool(name="pet", bufs=2, space=bass.MemorySpace.PSUM) as pet,
            tc.tile_pool(name="pvt", bufs=1, space=bass.MemorySpace.PSUM) as pvt,
        ):
            W_t = []
            for k in range(KT_D):
                t = cp.tile([128, O], RDT, tag=f"W{k}")
                nc.sync.dma_start(t[:], W_d.ap()[128 * k : 128 * (k + 1), :])
                W_t.append(t)
            dmask_t = cp.tile([128, O], F32, tag="dmask")
            nc.sync.dma_start(dmask_t[:], dmask_d.ap())
            vmask_t = cp.tile([128, G * 4 * NC_], F32, tag="vmask")
            nc.sync.dma_start(vmask_t[:], vmask_d.ap())
            ident_t = cp.tile([128, 128], F32, tag="ident")
            make_identity(nc, ident_t[:])
            magic_t = cp.tile([128, 1], U32, tag="magic")
            nc.vector.memset(magic_t[:], QMAGIC)
            c0_t = cp.tile([128, NC_], RDT, tag="c0")
            c0_f = cp.tile([128, NC_], F32, tag="c0f")
            nc.vector.memset(c0_f[:], 1.0 / NC_)
            nc.vector.tensor_copy(c0_t[:], c0_f[:])

            pools = (xp, up, utp, sp, ctp, pcre, pps, pet, pvt)
            consts = (W_t, dmask_t, vmask_t, ident_t, magic_t, c0_t)
            ngroups = n_ex // G

            def creation_gen(g):
                uh = [[None] * MT for _ in range(G)]
                uhT = [[None] * MT for _ in range(G)]
                gen = emit_creation(nc, pools, consts, xT_d.ap(), g, uh, uhT)
                return gen, (uh, uhT)

            cgen, made = creation_gen(0)
            for _ in cgen:
                pass
            for g in range(ngroups):
                rgen = emit_routing(nc, pools, consts, out_d.ap(), g, *made)
                if g + 1 < ngroups:
                    cgen, made = creation_gen(g + 1)
                else:
                    cgen = None
                for _ in rgen:
                    if cgen is not None:
                        next(cgen, None)
                if cgen is not None:
                    for _ in cgen:
                        pass

    nc.compile()
    return nc


_cache = {}


def _get_program():
    if "nc" not in _cache:
        _cache["nc"] = build()
    return _cache["nc"]


def _run(x: np.ndarray, W: np.ndarray, **spmd_kwargs):
    x = np.asarray(x, np.float32)
    W = np.asarray(W, np.float32)
    nc = _get_program()
    xT = np.ascontiguousarray(x.transpose(0, 2, 1))  # [B, D, S]
    dmask, vmask = host_masks()
    in_maps = []
    for c in range(N_CORES):
        in_maps.append(
            {
                "xT": xT[c * E : (c + 1) * E],
                "W": W,
                "dmask": dmask,
                "vmask": vmask,
            }
        )
    res = run_bass_kernel_spmd(
        nc, in_maps, core_ids=list(range(N_CORES)), **spmd_kwargs
    )
    out = np.concatenate([res.results[c]["out"] for c in range(N_CORES)], axis=0)
    return out, res


def kernel(x: np.ndarray, W: np.ndarray) -> np.ndarray:
    return _run(x, W)[0]


# revision 25
# speedup vs baseline: 1.0366x; 1.0366x over previous
"""CapsNet dynamic-routing layer on 8 Trainium2 NeuronCores.

Math (per example, S=512 input capsules of dim D=256, 16 output capsules of
dim 32, O = 16*32 = 512):
  u_hat = x @ W                     # [S, O]
  b = 0; for 3 routing iters:
    c = softmax_n(b)                # over the 16-capsule axis
    s[n] = sum_s c[n,s] * u_hat[s, n*32:(n+1)*32]
    v = s / sqrt(|s|^2 + 1e-7)
    b[n,s] = v[n] . u_hat[s, n*32:(n+1)*32]
  out = v.flatten()

Sharding: pure data-parallel over the batch (256 examples -> 32 per core),
W replicated, no cross-core communication.

Per-core structure: examples are processed in groups of 4 so that all the
thin [16, *] routing tensors pack into 32-partition strips of full
128-partition tiles (strip j holds example 4g+j; rows 16..31 of each strip
are dead). The four per-example routing matmuls of a K-tile go to four
different PE column groups (tile_position=(0, 32j)) and run concurrently.

Layouts (per example):
  u_hat  [S, O]  (S on partitions, 4 tiles) - rhs of the s-matmul
  u_hatT [O, S]  (O on partitions, 4 tiles) - rhs of the b-update matmul
Both come straight off the tensor engine from xT = x.T (host-transposed)
since both contract over D. b is kept transposed as bT [S, 16-per-ex] so
softmax runs along the free dim. Matmul operands use float32r (single-pass
fp32, ~1.6e-4 relative) unless use_f32r=False.
"""

import sys

sys.path.insert(0, "/opt/trn_rl_repo")

import numpy as np

import concourse.bacc as bacc
import concourse.mybir as mybir
import concourse.tile as tile
from concourse import bass
from concourse.bass_utils import run_bass_kernel_spmd
from concourse.masks import make_identity

F32 = mybir.dt.float32
F32R = mybir.dt.float32r
U32 = mybir.dt.uint32
AX = mybir.AxisListType
AF = mybir.ActivationFunctionType
OP = mybir.AluOpType

B, S, D = 256, 512, 256
NC_, DC = 16, 32  # num_capsule, dim_capsule
O = NC_ * DC  # 512
N_CORES = 8
E = B // N_CORES  # 32 examples per core
G = 4  # examples per group (one per PE column-group strip)
ROUTINGS = 3
KT_D = D // 128  # 2 k-tiles over D
MT = 4  # 4 tiles over S and over O
RDT = F32R  # matmul operand dtype (set by build())
QMAGIC = 0x5F3759DF  # rsqrt seed magic


def host_masks():
    # dmask4[32j+n, n'*32+d] = (n' == n) for n < 16, else 0 (strip pads dead)
    dmask4 = np.zeros((128, O), np.float32)
    for j in range(G):
        for n in range(NC_):
            dmask4[32 * j + n, n * DC : (n + 1) * DC] = 1.0
    # vmaskL[q, j*64 + k*16 + n'] = (n' == 4k + q//32), j-independent
    vmaskL = np.zeros((128, G * 4 * NC_), np.float32)
    for q in range(128):
        for j in range(G):
            for k in range(4):
                vmaskL[q, j * 64 + k * NC_ + 4 * k + q // 32] = 1.0
    return dmask4, vmaskL


def emit_creation(nc, pools, consts, xT_ap, g, uh, uhT):
    (xp, up, utp, sp, ctp, pcre, pps, pet, pvt) = pools
    (W_t, dmask_t, vmask_t, ident_t, magic_t, c0_t) = consts

    # ---- load xT for 4 examples: [D, (e, S)] as 2 partition tiles ----
    xt = []
    for k in range(KT_D):
        t = xp.tile([128, G, S], RDT, tag=f"xt{k}")
        nc.sync.dma_start(
            t[:],
            xT_ap[G * g : G * (g + 1), 128 * k : 128 * (k + 1), :].rearrange(
                "e p s -> p e s"
            ),
        )
        xt.append(t)

    # ---- u_hat [S, O] and u_hatT [O, S] per example ----
    for j in range(G):
        for m in range(MT):
            pu = pcre.tile([128, O], F32, tag="pcre")
            for k in range(KT_D):
                nc.tensor.matmul(
                    pu[:],
                    xt[k][:, j, bass.ts(m, 128)],
                    W_t[k][:],
                    start=(k == 0),
                    stop=(k == KT_D - 1),
                )
            t = up.tile([128, O], RDT, tag=f"uh{j}{m}")
            nc.scalar.copy(t[:], pu[:])
            uh[j][m] = t
        yield
        for m in range(MT):
            pu = pcre.tile([128, S], F32, tag="pcre")
            for k in range(KT_D):
                nc.tensor.matmul(
                    pu[:],
                    W_t[k][:, bass.ts(m, 128)],
                    xt[k][:, j, :],
                    start=(k == 0),
                    stop=(k == KT_D - 1),
                )
            t = utp.tile([128, S], RDT, tag=f"uht{j}{m}")
            nc.scalar.copy(t[:], pu[:])
            uhT[j][m] = t
        yield


def emit_routing(nc, pools, consts, out_ap, g, uh, uhT):
    (xp, up, utp, sp, ctp, pcre, pps, pet, pvt) = pools
    (W_t, dmask_t, vmask_t, ident_t, magic_t, c0_t) = consts

    # ---- routing (4 examples packed in 32-partition strips) ----
    cT = None  # [S-chunk m] -> [128, (j,16)] compact; iter 0 uses 1/16 const
    v = None
    for it in range(ROUTINGS):
        # s_full_j = cT_j.T @ u_hat_j : own [16, O] psum per example, then a
        # fused mask+gather packs the strips into one [128, O] sbuf tile
        # (fp32-family matmuls cannot write PSUM at partition offset != 0)
        masked = sp.tile([128, O], F32, tag="masked")
        nc.gpsimd.memset(masked[:], 0.0)
        for j in range(G):
            ps = pps.tile([NC_, O], F32, tag="ps")
            for m in range(MT):
                lhs = (
                    c0_t[:]
                    if cT is None
                    else cT[m][:, NC_ * j : NC_ * (j + 1)]
                )
                nc.tensor.matmul(
                    ps[:],
                    lhs,
                    uh[j][m][:],
                    start=(m == 0),
                    stop=(m == MT - 1),
                )
            nc.vector.tensor_mul(
                masked[32 * j : 32 * j + NC_, :], ps[:], dmask_t[: NC_, :]
            )
        yield
        s = sp.tile([128, DC], F32, tag="s")
        nc.vector.tensor_reduce(
            s[:],
            masked[:].rearrange("p (n d) -> p d n", n=NC_),
            axis=AX.X,
            op=OP.add,
        )
        # squash: v = s * rsqrt(|s|^2 + 1e-7); rsqrt = quake seed + 3 Newton
        sq = sp.tile([128, DC], F32, tag="sq")
        ss = sp.tile([128, 1], F32, tag="ss")
        nc.scalar.activation(sq[:], s[:], AF.Square, accum_out=ss[:])
        q = sp.tile([128, 1], F32, tag="q")
        nc.vector.tensor_scalar_add(q[:], ss[:], 1e-7)
        sh = sp.tile([128, 1], U32, tag="sh")
        nc.vector.tensor_scalar(
            sh[:], q[:].bitcast(U32), 1, None, op0=OP.logical_shift_right
        )
        y = sp.tile([128, 1], F32, tag="y")
        nc.vector.tensor_tensor(
            y[:].bitcast(U32), magic_t[:], sh[:], op=OP.subtract
        )
        for _ in range(2):
            t2 = sp.tile([128, 1], F32, tag="t2")
            nc.vector.tensor_tensor(t2[:], y[:], y[:], op=OP.mult)
            nc.vector.tensor_tensor(t2[:], t2[:], q[:], op=OP.mult)
            nc.vector.tensor_scalar(
                t2[:], t2[:], -0.5, 1.5, op0=OP.mult, op1=OP.add
            )
            nc.vector.tensor_tensor(y[:], y[:], t2[:], op=OP.mult)
        v = sp.tile([128, DC], F32, tag="v")
        nc.vector.tensor_scalar_mul(v[:], s[:], y[:])

        if it == ROUTINGS - 1:
            break

        # ---- b update: bT'[strip j] = Vblk_j.T @ u_hatT_j ----
        vtp = pvt.tile([DC, 128], F32, tag="vtp")
        nc.tensor.transpose(vtp[:], v[:], ident_t[:])
        vv = sp.tile([128, G * 4], F32, tag="vv")
        vtp_jx = vtp[:].rearrange("p (j x) -> p j x", j=G)
        for r in range(4):  # strip row n_lo = r: VV[32r+d,(j,k)] = vtp[d,32j+4k+r]
            nc.vector.tensor_copy(
                vv[32 * r : 32 * (r + 1), :].rearrange("p (j k) -> p j k", j=G),
                vtp_jx[:, :, r : NC_ : 4],
            )
        vblk = sp.tile([128, G * 4 * NC_], RDT, tag="vblk")
        nc.vector.tensor_mul(
            vblk[:].rearrange("p (j k n) -> p j k n", j=G, k=4),
            vmask_t[:].rearrange("p (j k n) -> p j k n", j=G, k=4),
            vv[:]
            .rearrange("p (j k one) -> p j k one", j=G, one=1)
            .to_broadcast([128, G, 4, NC_]),
        )
        # bT'_j = Vblk_j.T @ u_hatT_j in its own [16, S] psum; the per-strip
        # exp packs results into one [128, S] sbuf tile
        expb = sp.tile([128, S], F32, tag="expb")
        nc.gpsimd.memset(expb[:], 0.0)
        for j in range(G):
            pb = pps.tile([NC_, S], F32, tag="ps")
            for k in range(MT):
                nc.tensor.matmul(
                    pb[:],
                    vblk[:, 64 * j + NC_ * k : 64 * j + NC_ * (k + 1)],
                    uhT[j][k][:],
                    start=(k == 0),
                    stop=(k == MT - 1),
                )
            nc.scalar.activation(
                expb[32 * j : 32 * j + NC_, :], pb[:], AF.Exp
            )
        yield
        et = pet.tile([128, MT, 128], F32, tag="et")
        r_all = sp.tile([128, MT * G], F32, tag="r_all")
        for m in range(MT):
            nc.tensor.transpose(et[:, m, :], expb[:, bass.ts(m, 128)], ident_t[:])
            nc.vector.tensor_reduce(
                r_all[:, G * m : G * (m + 1)],
                et[:, m, :].rearrange("p (j n) -> p j n", j=G)[:, :, :NC_],
                axis=AX.X,
                op=OP.add,
            )
        rinv = sp.tile([128, MT * G], F32, tag="rinv")
        nc.vector.reciprocal(rinv[:], r_all[:])
        cT = []
        for m in range(MT):
            ct = ctp.tile([128, G * NC_], RDT, tag=f"ct{m}")
            nc.vector.tensor_mul(
                ct[:].rearrange("p (j n) -> p j n", j=G),
                et[:, m, :].rearrange("p (j n) -> p j n", j=G)[:, :, :NC_],
                rinv[:, G * m : G * (m + 1)]
                .rearrange("p (j one) -> p j one", one=1)
                .to_broadcast([128, G, NC_]),
            )
            cT.append(ct)

    # ---- output: strip j -> row 4g+j ----
    for j in range(G):
        nc.sync.dma_start(
            out_ap[G * g + j].rearrange("(n d) -> n d", n=NC_),
            v[32 * j : 32 * j + NC_, :],
        )


def build(n_ex=E, num_devices=N_CORES, use_f32r=True):
    global RDT
    RDT = F32R if use_f32r else F32
    assert n_ex % G == 0
    nc = bacc.Bacc(
        "TRN2", target_bir_lowering=False, debug=False, num_devices=num_devices
    )
    xT_d = nc.dram_tensor("xT", [n_ex, D, S], RDT, kind="ExternalInput")
    W_d = nc.dram_tensor("W", [D, O], RDT, kind="ExternalInput")
    dmask_d = nc.dram_tensor("dmask", [128, O], F32, kind="ExternalInput")
    vmask_d = nc.dram_tensor("vmask", [128, G * 4 * NC_], F32, kind="ExternalInput")
    out_d = nc.dram_tensor("out", [n_ex, O], F32, kind="ExternalOutput")

    with tile.TileContext(nc) as tc:
        with (
            tc.tile_pool(name="consts", bufs=1) as cp,
            tc.tile_pool(name="xp", bufs=2) as xp,
            tc.tile_pool(name="up", bufs=2) as up,
            tc.tile_pool(name="utp", bufs=2) as utp,
            tc.tile_pool(name="sp", bufs=4) as sp,
            tc.tile_pool(name="ctp", bufs=3) as ctp,
            tc.tile_pool(name="pcre", bufs=2, space=bass.MemorySpace.PSUM) as pcre,
            tc.tile_pool(name="pps", bufs=3, space=bass.MemorySpace.PSUM) as pps,
            tc.tile_pool(name="pet", bufs=2, space=bass.MemorySpace.PSUM) as pet,
            tc.tile_pool(name="pvt", bufs=1, space=bass.MemorySpace.PSUM) as pvt,
        ):
            W_t = []
            for k in range(KT_D):
                t = cp.tile([128, O], RDT, tag=f"W{k}")
                nc.sync.dma_start(t[:], W_d.ap()[128 * k : 128 * (k + 1), :])
                W_t.append(t)
            dmask_t = cp.tile([128, O], F32, tag="dmask")
            nc.sync.dma_start(dmask_t[:], dmask_d.ap())
            vmask_t = cp.tile([128, G * 4 * NC_], F32, tag="vmask")
            nc.sync.dma_start(vmask_t[:], vmask_d.ap())
            ident_t = cp.tile([128, 128], F32, tag="ident")
            make_identity(nc, ident_t[:])
            magic_t = cp.tile([128, 1], U32, tag="magic")
            nc.vector.memset(magic_t[:], QMAGIC)
            c0_t = cp.tile([128, NC_], RDT, tag="c0")
            c0_f = cp.tile([128, NC_], F32, tag="c0f")
            nc.vector.memset(c0_f[:], 1.0 / NC_)
            nc.vector.tensor_copy(c0_t[:], c0_f[:])

            pools = (xp, up, utp, sp, ctp, pcre, pps, pet, pvt)
            consts = (W_t, dmask_t, vmask_t, ident_t, magic_t, c0_t)
            ngroups = n_ex // G

            def creation_gen(g):
                uh = [[None] * MT for _ in range(G)]
                uhT = [[None] * MT for _ in range(G)]
                gen = emit_creation(nc, pools, consts, xT_d.ap(), g, uh, uhT)
                return gen, (uh, uhT)

            cgen, made = creation_gen(0)
            for _ in cgen:
                pass
            for g in range(ngroups):
                rgen = emit_routing(nc, pools, consts, out_d.ap(), g, *made)
                if g + 1 < ngroups:
                    cgen, made = creation_gen(g + 1)
                else:
                    cgen = None
                for _ in rgen:
                    if cgen is not None:
                        next(cgen, None)
                if cgen is not None:
                    for _ in cgen:
                        pass

    nc.compile()
    return nc


_cache = {}


def _get_program():
    if "nc" not in _cache:
        _cache["nc"] = build()
    return _cache["nc"]


def _run(x: np.ndarray, W: np.ndarray, **spmd_kwargs):
    x = np.asarray(x, np.float32)
    W = np.asarray(W, np.float32)
    nc = _get_program()
    xT = np.ascontiguousarray(x.transpose(0, 2, 1))  # [B, D, S]
    dmask, vmask = host_masks()
    in_maps = []
    for c in range(N_CORES):
        in_maps.append(
            {
                "xT": xT[c * E : (c + 1) * E],
                "W": W,
                "dmask": dmask,
                "vmask": vmask,
            }
        )
    res = run_bass_kernel_spmd(
        nc, in_maps, core_ids=list(range(N_CORES)), **spmd_kwargs
    )
    out = np.concatenate([res.results[c]["out"] for c in range(N_CORES)], axis=0)
    return out, res


def kernel(x: np.ndarray, W: np.ndarray) -> np.ndarray:
    return _run(x, W)[0]


# revision 26
# speedup vs baseline: 1.0382x; 1.0016x over previous
"""CapsNet dynamic-routing layer on 8 Trainium2 NeuronCores.

Math (per example, S=512 input capsules of dim D=256, 16 output capsules of
dim 32, O = 16*32 = 512):
  u_hat = x @ W                     # [S, O]
  b = 0; for 3 routing iters:
    c = softmax_n(b)                # over the 16-capsule axis
    s[n] = sum_s c[n,s] * u_hat[s, n*32:(n+1)*32]
    v = s / sqrt(|s|^2 + 1e-7)
    b[n,s] = v[n] . u_hat[s, n*32:(n+1)*32]
  out = v.flatten()

Sharding: pure data-parallel over the batch (256 examples -> 32 per core),
W replicated, no cross-core communication.

Per-core structure: examples are processed in groups of 4 so that all the
thin [16, *] routing tensors pack into 32-partition strips of full
128-partition tiles (strip j holds example 4g+j; rows 16..31 of each strip
are dead). The four per-example routing matmuls of a K-tile go to four
different PE column groups (tile_position=(0, 32j)) and run concurrently.

Layouts (per example):
  u_hat  [S, O]  (S on partitions, 4 tiles) - rhs of the s-matmul
  u_hatT [O, S]  (O on partitions, 4 tiles) - rhs of the b-update matmul
Both come straight off the tensor engine from xT = x.T (host-transposed)
since both contract over D. b is kept transposed as bT [S, 16-per-ex] so
softmax runs along the free dim. Matmul operands use float32r (single-pass
fp32, ~1.6e-4 relative) unless use_f32r=False.
"""

import sys

sys.path.insert(0, "/opt/trn_rl_repo")

import numpy as np

import concourse.bacc as bacc
import concourse.mybir as mybir
import concourse.tile as tile
from concourse import bass
from concourse.bass_utils import run_bass_kernel_spmd
from concourse.masks import make_identity

F32 = mybir.dt.float32
F32R = mybir.dt.float32r
U32 = mybir.dt.uint32
AX = mybir.AxisListType
AF = mybir.ActivationFunctionType
OP = mybir.AluOpType

B, S, D = 256, 512, 256
NC_, DC = 16, 32  # num_capsule, dim_capsule
O = NC_ * DC  # 512
N_CORES = 8
E = B // N_CORES  # 32 examples per core
G = 4  # examples per group (one per PE column-group strip)
ROUTINGS = 3
KT_D = D // 128  # 2 k-tiles over D
MT = 4  # 4 tiles over S and over O
RDT = F32R  # matmul operand dtype (set by build())
QMAGIC = 0x5F3759DF  # rsqrt seed magic


def host_masks():
    # dmask4[32j+n, n'*32+d] = (n' == n) for n < 16, else 0 (strip pads dead)
    dmask4 = np.zeros((128, O), np.float32)
    for j in range(G):
        for n in range(NC_):
            dmask4[32 * j + n, n * DC : (n + 1) * DC] = 1.0
    # vmaskL[q, j*64 + k*16 + n'] = (n' == 4k + q//32), j-independent
    vmaskL = np.zeros((128, G * 4 * NC_), np.float32)
    for q in range(128):
        for j in range(G):
            for k in range(4):
                vmaskL[q, j * 64 + k * NC_ + 4 * k + q // 32] = 1.0
    return dmask4, vmaskL


def emit_creation(nc, pools, consts, xT_ap, g, uh, uhT):
    (xp, up, utp, sp, ctp, pcre, pps, pet, pvt) = pools
    (W_t, dmask_t, vmask_t, ident_t, magic_t, c0_t) = consts

    # ---- load xT for 4 examples: [D, (e, S)] as 2 partition tiles ----
    xt = []
    for k in range(KT_D):
        t = xp.tile([128, G, S], RDT, tag=f"xt{k}")
        nc.sync.dma_start(
            t[:],
            xT_ap[G * g : G * (g + 1), 128 * k : 128 * (k + 1), :].rearrange(
                "e p s -> p e s"
            ),
        )
        xt.append(t)

    # ---- u_hat [S, O] and u_hatT [O, S] per example ----
    for j in range(G):
        for m in range(MT):
            pu = pcre.tile([128, O], F32, tag="pcre")
            for k in range(KT_D):
                nc.tensor.matmul(
                    pu[:],
                    xt[k][:, j, bass.ts(m, 128)],
                    W_t[k][:],
                    start=(k == 0),
                    stop=(k == KT_D - 1),
                )
            t = up.tile([128, O], RDT, tag=f"uh{j}{m}")
            nc.scalar.copy(t[:], pu[:])
            uh[j][m] = t
        yield
        for m in range(MT):
            pu = pcre.tile([128, S], F32, tag="pcre")
            for k in range(KT_D):
                nc.tensor.matmul(
                    pu[:],
                    W_t[k][:, bass.ts(m, 128)],
                    xt[k][:, j, :],
                    start=(k == 0),
                    stop=(k == KT_D - 1),
                )
            t = utp.tile([128, S], RDT, tag=f"uht{j}{m}")
            nc.scalar.copy(t[:], pu[:])
            uhT[j][m] = t
        yield


def emit_routing(nc, pools, consts, out_ap, g, uh, uhT):
    (xp, up, utp, sp, ctp, pcre, pps, pet, pvt) = pools
    (W_t, dmask_t, vmask_t, ident_t, magic_t, c0_t) = consts

    # ---- routing (4 examples packed in 32-partition strips) ----
    cT = None  # [S-chunk m] -> [128, (j,16)] compact; iter 0 uses 1/16 const
    v = None
    for it in range(ROUTINGS):
        # s_full_j = cT_j.T @ u_hat_j : own [16, O] psum per example, then a
        # fused mask+gather packs the strips into one [128, O] sbuf tile
        # (fp32-family matmuls cannot write PSUM at partition offset != 0)
        masked = sp.tile([128, O], F32, tag="masked")
        nc.gpsimd.memset(masked[:], 0.0)
        for j in range(G):
            ps = pps.tile([NC_, O], F32, tag="ps")
            for m in range(MT):
                lhs = (
                    c0_t[:]
                    if cT is None
                    else cT[m][:, NC_ * j : NC_ * (j + 1)]
                )
                nc.tensor.matmul(
                    ps[:],
                    lhs,
                    uh[j][m][:],
                    start=(m == 0),
                    stop=(m == MT - 1),
                )
            nc.vector.tensor_mul(
                masked[32 * j : 32 * j + NC_, :], ps[:], dmask_t[: NC_, :]
            )
        yield
        s = sp.tile([128, DC], F32, tag="s")
        nc.vector.tensor_reduce(
            s[:],
            masked[:].rearrange("p (n d) -> p d n", n=NC_),
            axis=AX.X,
            op=OP.add,
        )
        # squash: v = s * rsqrt(|s|^2 + 1e-7); rsqrt = quake seed + 3 Newton
        sq = sp.tile([128, DC], F32, tag="sq")
        ss = sp.tile([128, 1], F32, tag="ss")
        nc.scalar.activation(sq[:], s[:], AF.Square, accum_out=ss[:])
        q = sp.tile([128, 1], F32, tag="q")
        nc.vector.tensor_scalar_add(q[:], ss[:], 1e-7)
        sh = sp.tile([128, 1], U32, tag="sh")
        nc.vector.tensor_scalar(
            sh[:], q[:].bitcast(U32), 1, None, op0=OP.logical_shift_right
        )
        y = sp.tile([128, 1], F32, tag="y")
        nc.vector.tensor_tensor(
            y[:].bitcast(U32), magic_t[:], sh[:], op=OP.subtract
        )
        for _ in range(2):
            t2 = sp.tile([128, 1], F32, tag="t2")
            nc.vector.tensor_tensor(t2[:], y[:], y[:], op=OP.mult)
            nc.vector.tensor_tensor(t2[:], t2[:], q[:], op=OP.mult)
            nc.vector.tensor_scalar(
                t2[:], t2[:], -0.5, 1.5, op0=OP.mult, op1=OP.add
            )
            nc.vector.tensor_tensor(y[:], y[:], t2[:], op=OP.mult)
        v = sp.tile([128, DC], F32, tag="v")
        nc.vector.tensor_scalar_mul(v[:], s[:], y[:])

        if it == ROUTINGS - 1:
            break
        yield

        # ---- b update: bT'[strip j] = Vblk_j.T @ u_hatT_j ----
        vtp = pvt.tile([DC, 128], F32, tag="vtp")
        nc.tensor.transpose(vtp[:], v[:], ident_t[:])
        vv = sp.tile([128, G * 4], F32, tag="vv")
        vtp_jx = vtp[:].rearrange("p (j x) -> p j x", j=G)
        for r in range(4):  # strip row n_lo = r: VV[32r+d,(j,k)] = vtp[d,32j+4k+r]
            nc.vector.tensor_copy(
                vv[32 * r : 32 * (r + 1), :].rearrange("p (j k) -> p j k", j=G),
                vtp_jx[:, :, r : NC_ : 4],
            )
        vblk = sp.tile([128, G * 4 * NC_], RDT, tag="vblk")
        nc.vector.tensor_mul(
            vblk[:].rearrange("p (j k n) -> p j k n", j=G, k=4),
            vmask_t[:].rearrange("p (j k n) -> p j k n", j=G, k=4),
            vv[:]
            .rearrange("p (j k one) -> p j k one", j=G, one=1)
            .to_broadcast([128, G, 4, NC_]),
        )
        # bT'_j = Vblk_j.T @ u_hatT_j in its own [16, S] psum; the per-strip
        # exp packs results into one [128, S] sbuf tile
        expb = sp.tile([128, S], F32, tag="expb")
        nc.gpsimd.memset(expb[:], 0.0)
        for j in range(G):
            pb = pps.tile([NC_, S], F32, tag="ps")
            for k in range(MT):
                nc.tensor.matmul(
                    pb[:],
                    vblk[:, 64 * j + NC_ * k : 64 * j + NC_ * (k + 1)],
                    uhT[j][k][:],
                    start=(k == 0),
                    stop=(k == MT - 1),
                )
            nc.scalar.activation(
                expb[32 * j : 32 * j + NC_, :], pb[:], AF.Exp
            )
        yield
        et = pet.tile([128, MT, 128], F32, tag="et")
        r_all = sp.tile([128, MT * G], F32, tag="r_all")
        for m in range(MT):
            nc.tensor.transpose(et[:, m, :], expb[:, bass.ts(m, 128)], ident_t[:])
            nc.vector.tensor_reduce(
                r_all[:, G * m : G * (m + 1)],
                et[:, m, :].rearrange("p (j n) -> p j n", j=G)[:, :, :NC_],
                axis=AX.X,
                op=OP.add,
            )
        rinv = sp.tile([128, MT * G], F32, tag="rinv")
        nc.vector.reciprocal(rinv[:], r_all[:])
        cT = []
        for m in range(MT):
            ct = ctp.tile([128, G * NC_], RDT, tag=f"ct{m}")
            nc.vector.tensor_mul(
                ct[:].rearrange("p (j n) -> p j n", j=G),
                et[:, m, :].rearrange("p (j n) -> p j n", j=G)[:, :, :NC_],
                rinv[:, G * m : G * (m + 1)]
                .rearrange("p (j one) -> p j one", one=1)
                .to_broadcast([128, G, NC_]),
            )
            cT.append(ct)

    # ---- output: strip j -> row 4g+j ----
    for j in range(G):
        nc.sync.dma_start(
            out_ap[G * g + j].rearrange("(n d) -> n d", n=NC_),
            v[32 * j : 32 * j + NC_, :],
        )


def build(n_ex=E, num_devices=N_CORES, use_f32r=True):
    global RDT
    RDT = F32R if use_f32r else F32
    assert n_ex % G == 0
    nc = bacc.Bacc(
        "TRN2", target_bir_lowering=False, debug=False, num_devices=num_devices
    )
    xT_d = nc.dram_tensor("xT", [n_ex, D, S], RDT, kind="ExternalInput")
    W_d = nc.dram_tensor("W", [D, O], RDT, kind="ExternalInput")
    dmask_d = nc.dram_tensor("dmask", [128, O], F32, kind="ExternalInput")
    vmask_d = nc.dram_tensor("vmask", [128, G * 4 * NC_], F32, kind="ExternalInput")
    out_d = nc.dram_tensor("out", [n_ex, O], F32, kind="ExternalOutput")

    with tile.TileContext(nc) as tc:
        with (
            tc.tile_pool(name="consts", bufs=1) as cp,
            tc.tile_pool(name="xp", bufs=2) as xp,
            tc.tile_pool(name="up", bufs=2) as up,
            tc.tile_pool(name="utp", bufs=2) as utp,
            tc.tile_pool(name="sp", bufs=4) as sp,
            tc.tile_pool(name="ctp", bufs=3) as ctp,
            tc.tile_pool(name="pcre", bufs=2, space=bass.MemorySpace.PSUM) as pcre,
            tc.tile_pool(name="pps", bufs=3, space=bass.MemorySpace.PSUM) as pps,
            tc.tile_pool(name="pet", bufs=2, space=bass.MemorySpace.PSUM) as pet,
            tc.tile_pool(name="pvt", bufs=1, space=bass.MemorySpace.PSUM) as pvt,
        ):
            W_t = []
            for k in range(KT_D):
                t = cp.tile([128, O], RDT, tag=f"W{k}")
                nc.sync.dma_start(t[:], W_d.ap()[128 * k : 128 * (k + 1), :])
                W_t.append(t)
            dmask_t = cp.tile([128, O], F32, tag="dmask")
            nc.sync.dma_start(dmask_t[:], dmask_d.ap())
            vmask_t = cp.tile([128, G * 4 * NC_], F32, tag="vmask")
            nc.sync.dma_start(vmask_t[:], vmask_d.ap())
            ident_t = cp.tile([128, 128], F32, tag="ident")
            make_identity(nc, ident_t[:])
            magic_t = cp.tile([128, 1], U32, tag="magic")
            nc.vector.memset(magic_t[:], QMAGIC)
            c0_t = cp.tile([128, NC_], RDT, tag="c0")
            c0_f = cp.tile([128, NC_], F32, tag="c0f")
            nc.vector.memset(c0_f[:], 1.0 / NC_)
            nc.vector.tensor_copy(c0_t[:], c0_f[:])

            pools = (xp, up, utp, sp, ctp, pcre, pps, pet, pvt)
            consts = (W_t, dmask_t, vmask_t, ident_t, magic_t, c0_t)
            ngroups = n_ex // G

            def creation_gen(g):
                uh = [[None] * MT for _ in range(G)]
                uhT = [[None] * MT for _ in range(G)]
                gen = emit_creation(nc, pools, consts, xT_d.ap(), g, uh, uhT)
                return gen, (uh, uhT)

            cgen, made = creation_gen(0)
            for _ in cgen:
                pass
            for g in range(ngroups):
                rgen = emit_routing(nc, pools, consts, out_d.ap(), g, *made)
                if g + 1 < ngroups:
                    cgen, made = creation_gen(g + 1)
                else:
                    cgen = None
                for _ in rgen:
                    if cgen is not None:
                        next(cgen, None)
                if cgen is not None:
                    for _ in cgen:
                        pass

    nc.compile()
    return nc


_cache = {}


def _get_program():
    if "nc" not in _cache:
        _cache["nc"] = build()
    return _cache["nc"]


def _run(x: np.ndarray, W: np.ndarray, **spmd_kwargs):
    x = np.asarray(x, np.float32)
    W = np.asarray(W, np.float32)
    nc = _get_program()
    xT = np.ascontiguousarray(x.transpose(0, 2, 1))  # [B, D, S]
    dmask, vmask = host_masks()
    in_maps = []
    for c in range(N_CORES):
        in_maps.append(
            {
                "xT": xT[c * E : (c + 1) * E],
                "W": W,
                "dmask": dmask,
                "vmask": vmask,
            }
        )
    res = run_bass_kernel_spmd(
        nc, in_maps, core_ids=list(range(N_CORES)), **spmd_kwargs
    )
    out = np.concatenate([res.results[c]["out"] for c in range(N_CORES)], axis=0)
    return out, res


def kernel(x: np.ndarray, W: np.ndarray) -> np.ndarray:
    return _run(x, W)[0]
